# revision 5
# baseline (speedup 1.0000x reference)
"""Distributed Trainium2 Bass kernel for nn_AdjConv (gnn_message_passing).

Full (unsharded) inputs in, full output out. Internally shards the vertex
dim N=4096 across 8 NeuronCores (512 rows each); hyperedge dim E=1024 is
local to every core.

Math (see reference): with LN invariant to positive row scaling, the
softmax denominator and the /adj.sum(0) division cancel inside the two
LayerNorms, so the on-chip pipeline is:

  raw   = feats.T @ adj                      (AllReduce over row shards)
  fT    = (feats @ W_v.T).T                  (AllGather, bf16)
  esT   = exp((f f.T)/8).T  row-shard        (no max-subtract needed)
  dT    = LN_h(esT.T @ f).T * ln2w + ln2b    (partition-dim stats via ones-matmuls)
  sT    = LN_h(raw.T @ lin.T).T * ln1w + ln1b
  ta    = exp((2(w*s).T d - dd)/800 - (ss+b)/800)   (E x n_local, e on partitions)
  DV    = 1.ta (local), DE = ta.1 (AllReduce)
  BT    = ta * invDV[col]; AT = BT * 0.01*invDE[row]  (AllGather BT)
  out   = 0.99*G + AT.T @ BT                 (G pre-scaled by 0.99 on host)
"""
import numpy as np
import ml_dtypes

import concourse.bass as bass
import concourse.bacc as bacc
import concourse.mybir as mybir
from concourse import tile
from concourse.bass_utils import run_bass_kernel_spmd

BF = ml_dtypes.bfloat16
F32 = np.float32
DT_BF = mybir.dt.bfloat16
DT_F32 = mybir.dt.float32
MULT = mybir.AluOpType.mult
ADD = mybir.AluOpType.add
EXP = mybir.ActivationFunctionType.Exp
SQRT = mybir.ActivationFunctionType.Sqrt
SQUARE = mybir.ActivationFunctionType.Square
IDENT_F = mybir.ActivationFunctionType.Identity

N, E, D, H = 4096, 1024, 256, 64
NC = 8          # cores
NL = N // NC    # 512 local rows
P = 128
NKT = NL // P   # 4  local-row partition tiles
EKT = E // P    # 8  e-chunks
DKT = D // P    # 2  d-chunks
KT = N // P     # 32 n' tiles
NB = 512        # output column block
NBT = N // NB   # 8

LN_EPS = 1e-5


def build_kernel():
    nc = bacc.Bacc("TRN2", target_bir_lowering=False, debug=False,
                   num_devices=NC)

    # ---- per-core external I/O -------------------------------------------
    adj_e = nc.dram_tensor("adj", [NL, E], DT_BF, kind="ExternalInput")
    g_e = nc.dram_tensor("g", [NL, N], DT_F32, kind="ExternalInput")
    feats_e = nc.dram_tensor("feats", [NL, D], DT_BF, kind="ExternalInput")
    featsT_e = nc.dram_tensor("featsT", [D, NL], DT_BF, kind="ExternalInput")
    wvT_e = nc.dram_tensor("wvT", [D, H], DT_BF, kind="ExternalInput")
    linT_e = nc.dram_tensor("linT", [D, H], DT_BF, kind="ExternalInput")
    wcol_e = nc.dram_tensor("wcol", [H, 1], DT_BF, kind="ExternalInput")
    w2col_e = nc.dram_tensor("w2col", [H, 1], DT_F32, kind="ExternalInput")
    ln1_e = nc.dram_tensor("ln1", [H, 2], DT_F32, kind="ExternalInput")
    ln2_e = nc.dram_tensor("ln2", [H, 2], DT_F32, kind="ExternalInput")
    negb_e = nc.dram_tensor("negb800", [P, 1], DT_F32, kind="ExternalInput")
    ident_e = nc.dram_tensor("ident", [P, P], DT_BF, kind="ExternalInput")
    out_e = nc.dram_tensor("out", [NL, N], DT_F32, kind="ExternalOutput")

    # ---- internal DRAM (collective bounce buffers) -----------------------
    ar1_in = nc.dram_tensor("ar1_in", [D, E], DT_BF)
    ar1_out = nc.dram_tensor("ar1_out", [D, E], DT_BF, addr_space="Shared")
    agf_in = nc.dram_tensor("agf_in", [H, NL], DT_BF)
    agf_out = nc.dram_tensor("agf_out", [NC, H, NL], DT_BF, addr_space="Shared")
    arde_in = nc.dram_tensor("arde_in", [P, EKT], DT_F32)
    arde_out = nc.dram_tensor("arde_out", [P, EKT], DT_F32, addr_space="Shared")
    agb_in = nc.dram_tensor("agb_in", [EKT, P, NL], DT_BF)
    agb_out = nc.dram_tensor("agb_out", [NC, EKT, P, NL], DT_BF,
                             addr_space="Shared")

    rg = [list(range(NC))]

    with tile.TileContext(nc) as tc:
        with (
            tc.tile_pool(name="pers", bufs=1) as pers,
            tc.tile_pool(name="gio", bufs=3) as gio,
        ):
            def ptile(shape, dt, tag, bufs=None):
                return pers.tile(shape, dt, tag=tag, name=tag, bufs=bufs)

            # ---- load inputs into SBUF ----------------------------------
            adj_sb = []
            feats_sb = []
            for k in range(NKT):
                t = ptile([P, E], DT_BF, f"adj{k}")
                nc.sync.dma_start(out=t[:], in_=adj_e[k * P:(k + 1) * P, :])
                adj_sb.append(t)
                t = ptile([P, D], DT_BF, f"feats{k}")
                nc.sync.dma_start(out=t[:], in_=feats_e[k * P:(k + 1) * P, :])
                feats_sb.append(t)
            featsT_sb = []
            wvT_sb = []
            linT_sb = []
            for k in range(DKT):
                t = ptile([P, NL], DT_BF, f"featsT{k}")
                nc.sync.dma_start(out=t[:], in_=featsT_e[k * P:(k + 1) * P, :])
                featsT_sb.append(t)
                t = ptile([P, H], DT_BF, f"wvT{k}")
                nc.sync.dma_start(out=t[:], in_=wvT_e[k * P:(k + 1) * P, :])
                wvT_sb.append(t)
                t = ptile([P, H], DT_BF, f"linT{k}")
                nc.sync.dma_start(out=t[:], in_=linT_e[k * P:(k + 1) * P, :])
                linT_sb.append(t)
            wcol = ptile([H, 1], DT_BF, "wcol")
            nc.sync.dma_start(out=wcol[:], in_=wcol_e[:, :])
            w2col = ptile([H, 1], DT_F32, "w2col")
            nc.sync.dma_start(out=w2col[:], in_=w2col_e[:, :])
            ln1 = ptile([H, 2], DT_F32, "ln1")
            nc.sync.dma_start(out=ln1[:], in_=ln1_e[:, :])
            ln2 = ptile([H, 2], DT_F32, "ln2")
            nc.sync.dma_start(out=ln2[:], in_=ln2_e[:, :])
            negb = ptile([P, 1], DT_F32, "negb")
            nc.sync.dma_start(out=negb[:], in_=negb_e[:, :])
            ident = ptile([P, P], DT_BF, "ident")
            nc.sync.dma_start(out=ident[:], in_=ident_e[:, :])

            ones_col = ptile([P, 1], DT_BF, "ones_col")
            nc.vector.memset(ones_col[:], 1.0)
            ones_row = ptile([1, P], DT_BF, "ones_row")
            nc.vector.memset(ones_row[:], 1.0)
            neg_row = ptile([1, P], DT_BF, "neg_row")
            nc.vector.memset(neg_row[:], -1.0)
            eps_col = ptile([P, 1], DT_F32, "eps_col")
            nc.vector.memset(eps_col[:], LN_EPS)

            with (
                tc.tile_pool(name="psA", bufs=1, space="PSUM") as psA,
                tc.tile_pool(name="escp", bufs=1) as escp,
            ):
                def atile(shape, dt, tag, bufs=1):
                    return psA.tile(shape, dt, tag=tag, name=tag, bufs=bufs)

                # ---- phase 1: raw partial = feats_l.T @ adj_l -> AR ------
                for dc in range(DKT):
                    for eh in range(2):
                        ps = atile([P, 512], DT_F32, "ec", bufs=2)
                        for k in range(NKT):
                            nc.tensor.matmul(
                                ps[:],
                                lhsT=feats_sb[k][:, dc * P:(dc + 1) * P],
                                rhs=adj_sb[k][:, eh * 512:(eh + 1) * 512],
                                start=(k == 0), stop=(k == NKT - 1))
                        sb = ptile([P, 512], DT_BF, f"ecs{dc}{eh}")
                        nc.scalar.copy(sb[:], ps[:])
                        nc.sync.dma_start(
                            out=ar1_in[dc * P:(dc + 1) * P,
                                       eh * 512:(eh + 1) * 512],
                            in_=sb[:])
                nc.gpsimd.collective_compute(
                    "AllReduce", mybir.AluOpType.add, replica_groups=rg,
                    ins=[ar1_in[:, :]], outs=[ar1_out[:, :]])

                # ---- phase 2: fT_loc = wvT.T @ featsT -> AllGather -------
                ps_f = atile([H, NL], DT_F32, "sm", bufs=2)
                for k in range(DKT):
                    nc.tensor.matmul(ps_f[:], lhsT=wvT_sb[k][:],
                                     rhs=featsT_sb[k][:],
                                     start=(k == 0), stop=(k == DKT - 1))
                fT_loc = ptile([H, NL], DT_BF, "fT_loc")
                nc.scalar.copy(fT_loc[:], ps_f[:])
                nc.sync.dma_start(out=agf_in[:, :], in_=fT_loc[:])
                nc.gpsimd.collective_compute(
                    "AllGather", mybir.AluOpType.bypass, replica_groups=rg,
                    ins=[agf_in[:, :]], outs=[agf_out[:, :, :]])

                # assemble fT_full (H, N); f_nat tiles (P, H) via transpose
                fT_full = ptile([H, N], DT_BF, "fT_full")
                for r in range(NC):
                    nc.sync.dma_start(out=fT_full[:, r * NL:(r + 1) * NL],
                                      in_=agf_out[r, :, :])
                f_nat = []
                for k in range(KT):
                    pt = atile([P, H], DT_BF, "smb", bufs=1)
                    nc.tensor.transpose(pt[:], fT_full[:, k * P:(k + 1) * P],
                                        ident[:H, :H])
                    t = ptile([P, H], DT_BF, f"fnat{k}")
                    nc.scalar.copy(t[:], pt[:])
                    f_nat.append(t)

                # ---- phase 3: expscoresT + dT accumulation ---------------
                ps_dT = atile([H, NL], DT_F32, "dT")
                esc = []
                for k in range(KT):
                    ps = atile([P, NL], DT_F32, "sc", bufs=2)
                    nc.tensor.matmul(ps[:], lhsT=fT_full[:, k * P:(k + 1) * P],
                                     rhs=fT_loc[:], start=True, stop=True)
                    es = escp.tile([P, NL], DT_BF, tag=f"esc{k}",
                                   name=f"esc{k}")
                    nc.scalar.activation(es[:], ps[:], EXP, scale=0.125)
                    esc.append(es)
                    nc.tensor.matmul(ps_dT[:], lhsT=f_nat[k][:], rhs=es[:],
                                     start=(k == 0), stop=(k == KT - 1))

                # dT LayerNorm: stats along partition dim via ones-matmuls
                dT_pre = ptile([H, NL], DT_BF, "dT_pre")
                nc.vector.tensor_copy(dT_pre[:], ps_dT[:])
                d2 = ptile([H, NL], DT_BF, "d2tmp")
                nc.vector.tensor_mul(d2[:], dT_pre[:], dT_pre[:])
                ps_srow = atile([1, NL], DT_F32, "sm", bufs=2)
                nc.tensor.matmul(ps_srow[:], lhsT=ones_col[:H, :],
                                 rhs=dT_pre[:], start=True, stop=True)
                ps_sqrow = atile([1, NL], DT_F32, "sm", bufs=2)
                nc.tensor.matmul(ps_sqrow[:], lhsT=ones_col[:H, :], rhs=d2[:],
                                 start=True, stop=True)
                mean_r = ptile([1, NL], DT_F32, "mean_r")
                nc.scalar.mul(mean_r[:], ps_srow[:], 1.0 / H)
                msq_r = ptile([1, NL], DT_F32, "msq_r")
                nc.vector.tensor_mul(msq_r[:], mean_r[:], mean_r[:])
                var_r = ptile([1, NL], DT_F32, "var_r")
                nc.scalar.mul(var_r[:], ps_sqrow[:], 1.0 / H)
                nc.vector.tensor_sub(var_r[:], var_r[:], msq_r[:])
                sd_r = ptile([1, NL], DT_F32, "sd_r")
                nc.scalar.activation(sd_r[:], var_r[:], SQRT,
                                     bias=eps_col[:1, :])
                rstd_r = ptile([1, NL], DT_F32, "rstd_r")
                nc.vector.reciprocal(rstd_r[:], sd_r[:])
                ab_row = ptile([1, 2 * NL], DT_BF, "ab_row")
                nc.vector.tensor_copy(ab_row[:, 0:NL], rstd_r[:])
                nmr_r = ptile([1, NL], DT_F32, "nmr_r")
                nc.vector.tensor_mul(nmr_r[:], mean_r[:], rstd_r[:])
                nc.vector.tensor_scalar(ab_row[:, NL:2 * NL], nmr_r[:], -1.0,
                                        None, MULT)
                # broadcast A,B rows to H partitions
                ab_a = atile([H, NL], DT_F32, "sm", bufs=2)
                nc.tensor.matmul(ab_a[:], lhsT=ones_row[:, :H],
                                 rhs=ab_row[:, 0:NL], start=True, stop=True)
                ab_b = atile([H, NL], DT_F32, "smb", bufs=1)
                nc.tensor.matmul(ab_b[:], lhsT=ones_row[:, :H],
                                 rhs=ab_row[:, NL:2 * NL], start=True,
                                 stop=True)
                ab_bc = ptile([H, 2 * NL], DT_BF, "ab_bc")
                nc.vector.tensor_copy(ab_bc[:, 0:NL], ab_a[:])
                nc.vector.tensor_copy(ab_bc[:, NL:2 * NL], ab_b[:])
                t1 = ptile([H, NL], DT_F32, "dnorm_t1")
                nc.vector.tensor_mul(t1[:], dT_pre[:], ab_bc[:, 0:NL])
                nc.vector.tensor_add(t1[:], t1[:], ab_bc[:, NL:2 * NL])
                dT_ln = ptile([H, NL], DT_BF, "dT_ln")
                nc.vector.tensor_scalar(dT_ln[:], t1[:], ln2[:, 0:1],
                                        ln2[:, 1:2], MULT, ADD)
                d2T = ptile([H, NL], DT_BF, "d2T")
                nc.vector.tensor_mul(d2T[:], dT_ln[:], dT_ln[:])
                ps_dd = atile([1, NL], DT_F32, "sm", bufs=2)
                nc.tensor.matmul(ps_dd[:], lhsT=wcol[:], rhs=d2T[:],
                                 start=True, stop=True)
                dd_bf = ptile([1, NL], DT_BF, "dd_bf")
                nc.scalar.copy(dd_bf[:], ps_dd[:])

                # ---- phase 4: s side (after AllReduce #1) ----------------
                raw_bf = []
                for k in range(DKT):
                    t = ptile([P, E], DT_BF, f"raw{k}")
                    nc.sync.dma_start(out=t[:],
                                      in_=ar1_out[k * P:(k + 1) * P, :])
                    raw_bf.append(t)
                sT_ln = ptile([H, E], DT_BF, "sT_ln")
                for ec in range(EKT):
                    ps = atile([P, H], DT_F32, "sm", bufs=2)
                    for k in range(DKT):
                        nc.tensor.matmul(
                            ps[:], lhsT=raw_bf[k][:, ec * P:(ec + 1) * P],
                            rhs=linT_sb[k][:],
                            start=(k == 0), stop=(k == DKT - 1))
                    ssum = ptile([P, 1], DT_F32, "ln_ssum", bufs=2)
                    nc.vector.reduce_sum(ssum[:], ps[:],
                                         axis=mybir.AxisListType.X)
                    nmean = ptile([P, 1], DT_F32, "ln_nmean", bufs=2)
                    nc.scalar.mul(nmean[:], ssum[:], -1.0 / H)
                    xc = ptile([P, H], DT_F32, "ln_xc", bufs=2)
                    nc.scalar.activation(xc[:], ps[:], IDENT_F,
                                         bias=nmean[:], scale=1.0)
                    sq = ptile([P, H], DT_F32, "ln_sq", bufs=2)
                    vsum = ptile([P, 1], DT_F32, "ln_vsum", bufs=2)
                    nc.scalar.activation(sq[:], xc[:], SQUARE,
                                         accum_out=vsum[:])
                    sd = ptile([P, 1], DT_F32, "ln_sd", bufs=2)
                    nc.scalar.activation(sd[:], vsum[:], SQRT,
                                         scale=1.0 / H, bias=eps_col[:])
                    rstd = ptile([P, 1], DT_F32, "ln_rstd", bufs=2)
                    nc.vector.reciprocal(rstd[:], sd[:])
                    snrm = ptile([P, H], DT_BF, "ln_snrm", bufs=2)
                    nc.vector.tensor_scalar(snrm[:], xc[:], rstd[:], None,
                                            MULT)
                    pt = atile([H, P], DT_BF, "smb", bufs=1)
                    nc.tensor.transpose(pt[:], snrm[:], ident[:])
                    nc.vector.tensor_scalar(sT_ln[:, ec * P:(ec + 1) * P],
                                            pt[:], ln1[:, 0:1], ln1[:, 1:2],
                                            MULT, ADD)

            sT2w = ptile([H, E], DT_BF, "sT2w")
            nc.vector.tensor_scalar(sT2w[:], sT_ln[:], w2col[:], None, MULT)
            s2T = ptile([H, E], DT_BF, "s2T")
            nc.vector.tensor_mul(s2T[:], sT_ln[:], sT_ln[:])

            # ---- phase 5: ta tiles, DV, DE ------------------------------
            with tc.tile_pool(name="psB", bufs=1, space="PSUM") as psB:
                def btile(shape, dt, tag, bufs=1):
                    return psB.tile(shape, dt, tag=tag, name=tag, bufs=bufs)

                bias_sb = ptile([P, EKT], DT_F32, "bias_sb")
                de_cols = ptile([P, EKT], DT_F32, "de_cols")
                ta = []
                for ec in range(EKT):
                    ps_ss = btile([P, 1], DT_F32, "ss", bufs=2)
                    nc.tensor.matmul(ps_ss[:],
                                     lhsT=s2T[:, ec * P:(ec + 1) * P],
                                     rhs=wcol[:], start=True, stop=True)
                    nc.vector.scalar_tensor_tensor(
                        bias_sb[:, ec:ec + 1], ps_ss[:], -1.0 / 800.0,
                        negb[:], MULT, ADD)
                    ps = btile([P, NL], DT_F32, "ta", bufs=2)
                    nc.tensor.matmul(ps[:], lhsT=sT2w[:, ec * P:(ec + 1) * P],
                                     rhs=dT_ln[:], start=True, stop=False)
                    nc.tensor.matmul(ps[:], lhsT=neg_row[:], rhs=dd_bf[:],
                                     start=False, stop=True)
                    t = ptile([P, NL], DT_BF, f"ta{ec}")
                    nc.scalar.activation(t[:], ps[:], EXP, scale=1.0 / 800.0,
                                         bias=bias_sb[:, ec:ec + 1],
                                         accum_out=de_cols[:, ec:ec + 1])
                    ta.append(t)

                # DE AllReduce
                nc.sync.dma_start(out=arde_in[:, :], in_=de_cols[:])
                nc.gpsimd.collective_compute(
                    "AllReduce", mybir.AluOpType.add, replica_groups=rg,
                    ins=[arde_in[:, :]], outs=[arde_out[:, :]])

                # DV (local): column sums over all e
                ps_dv = btile([1, NL], DT_F32, "dv")
                for ec in range(EKT):
                    nc.tensor.matmul(ps_dv[:], lhsT=ones_col[:], rhs=ta[ec][:],
                                     start=(ec == 0), stop=(ec == EKT - 1))
                rdv = ptile([1, NL], DT_F32, "rdv")
                nc.vector.reciprocal(rdv[:], ps_dv[:])
                invdv_row = ptile([1, NL], DT_BF, "invdv_row")
                nc.scalar.activation(invdv_row[:], rdv[:], SQRT)
                ps_bc = btile([P, NL], DT_F32, "dvbc")
                nc.tensor.matmul(ps_bc[:], lhsT=ones_row[:], rhs=invdv_row[:],
                                 start=True, stop=True)
                invdv_bc = ptile([P, NL], DT_BF, "invdv_bc")
                nc.vector.tensor_copy(invdv_bc[:], ps_bc[:])

                # BT tiles -> bounce -> AllGather
                bt = []
                for ec in range(EKT):
                    t = ptile([P, NL], DT_BF, f"bt{ec}")
                    nc.vector.tensor_mul(t[:], ta[ec][:], invdv_bc[:])
                    nc.sync.dma_start(out=agb_in[ec, :, :], in_=t[:])
                    bt.append(t)
                nc.gpsimd.collective_compute(
                    "AllGather", mybir.AluOpType.bypass, replica_groups=rg,
                    ins=[agb_in[:, :, :]], outs=[agb_out[:, :, :, :]])

                # invDE from AllReduce; AT = BT * (0.01*invDE)
                de_sb = ptile([P, EKT], DT_F32, "de_sb")
                nc.sync.dma_start(out=de_sb[:], in_=arde_out[:, :])
                invde = ptile([P, EKT], DT_F32, "invde")
                nc.vector.reciprocal(invde[:], de_sb[:])
                invde01 = ptile([P, EKT], DT_F32, "invde01")
                nc.vector.tensor_scalar(invde01[:], invde[:], 0.01, None,
                                        MULT)
                at = []
                for ec in range(EKT):
                    t = ptile([P, NL], DT_BF, f"at{ec}")
                    nc.vector.tensor_scalar(t[:], bt[ec][:],
                                            invde01[:, ec:ec + 1], None, MULT)
                    at.append(t)

            # ---- phase 6: big matmul + epilogue -------------------------
            with (
                tc.tile_pool(name="psC", bufs=8, space="PSUM") as psC,
                tc.tile_pool(name="btfp", bufs=1) as btfp,
            ):
                btf = []
                for k in range(EKT):
                    t = btfp.tile([P, N], DT_BF, tag=f"btf{k}", name=f"btf{k}")
                    for r in range(NC):
                        nc.sync.dma_start(out=t[:, r * NL:(r + 1) * NL],
                                          in_=agb_out[r, k, :, :])
                    btf.append(t)

                for m in range(NKT):
                    pss = []
                    for nb in range(NBT):
                        pss.append(psC.tile([P, NB], DT_F32, tag="big",
                                            name="big"))
                    for k in range(EKT):
                        for nb in range(NBT):
                            nc.tensor.matmul(
                                pss[nb][:],
                                lhsT=at[k][:, m * P:(m + 1) * P],
                                rhs=btf[k][:, nb * NB:(nb + 1) * NB],
                                start=(k == 0), stop=(k == EKT - 1))
                    for nb in range(NBT):
                        gsb = gio.tile([P, NB], DT_F32, tag="gsb", name="gsb")
                        nc.sync.dma_start(
                            out=gsb[:],
                            in_=g_e[m * P:(m + 1) * P, nb * NB:(nb + 1) * NB])
                        osb = gio.tile([P, NB], DT_F32, tag="osb", name="osb")
                        nc.vector.tensor_add(osb[:], gsb[:], pss[nb][:])
                        nc.sync.dma_start(
                            out=out_e[m * P:(m + 1) * P,
                                      nb * NB:(nb + 1) * NB],
                            in_=osb[:])

    nc.compile()
    return nc


_NC_CACHE = None


def _get_nc():
    global _NC_CACHE
    if _NC_CACHE is None:
        _NC_CACHE = build_kernel()
    return _NC_CACHE


def make_in_maps(adj, G, feats, W_v_w, lin_w, w_o_w, w_o_b,
                 ln1_w, ln1_b, ln2_w, ln2_b, kn=None):
    adj = np.asarray(adj, F32)
    G = np.asarray(G, F32)
    feats = np.asarray(feats, F32)
    W_v_w = np.asarray(W_v_w, F32)
    lin_w = np.asarray(lin_w, F32)
    w = np.asarray(w_o_w, F32)[0]
    b = float(np.asarray(w_o_b, F32).reshape(-1)[0])
    ln1_w = np.asarray(ln1_w, F32).reshape(-1)
    ln1_b = np.asarray(ln1_b, F32).reshape(-1)
    ln2_w = np.asarray(ln2_w, F32).reshape(-1)
    ln2_b = np.asarray(ln2_b, F32).reshape(-1)

    g99 = G * np.float32(0.99)
    adj_bf = adj.astype(BF)
    feats_bf = feats.astype(BF)
    featsT_bf = np.ascontiguousarray(feats.T).astype(BF)
    wvT = np.ascontiguousarray(W_v_w.T).astype(BF)
    linT = np.ascontiguousarray(lin_w.T).astype(BF)
    wcol = np.ascontiguousarray(w.reshape(H, 1)).astype(BF)
    w2col = np.ascontiguousarray((2.0 * w).reshape(H, 1)).astype(F32)
    ln1 = np.stack([ln1_w, ln1_b], axis=1).astype(F32)
    ln2 = np.stack([ln2_w, ln2_b], axis=1).astype(F32)
    negb = np.full((P, 1), -b / 800.0, F32)
    ident = np.eye(P, dtype=BF)

    in_maps = []
    for i in range(NC):
        sl = slice(i * NL, (i + 1) * NL)
        in_maps.append({
            "adj": np.ascontiguousarray(adj_bf[sl]),
            "g": np.ascontiguousarray(g99[sl]),
            "feats": np.ascontiguousarray(feats_bf[sl]),
            "featsT": np.ascontiguousarray(featsT_bf[:, sl]),
            "wvT": wvT,
            "linT": linT,
            "wcol": wcol,
            "w2col": w2col,
            "ln1": ln1,
            "ln2": ln2,
            "negb800": negb,
            "ident": ident,
        })
    return in_maps


def kernel(**inputs) -> np.ndarray:
    nc = _get_nc()
    in_maps = make_in_maps(**inputs)
    res = run_bass_kernel_spmd(nc, in_maps, core_ids=list(range(NC))).results
    return np.concatenate([np.asarray(res[i]["out"]) for i in range(NC)],
                          axis=0)


if __name__ == "__main__":
    import reference
    inputs = reference.setup_inputs()
    out = kernel(**{k: np.asarray(v) if not np.isscalar(v) else v
                    for k, v in inputs.items()})
    print("out", out.shape, out.dtype)


# revision 9
# speedup vs baseline: 1.0504x; 1.0504x over previous
"""Distributed Trainium2 Bass kernel for nn_AdjConv (gnn_message_passing).

Full (unsharded) inputs in, full output out. Internally shards the vertex
dim N=4096 across 8 NeuronCores (512 rows each); hyperedge dim E=1024 is
local to every core.

Math (see reference): with LN invariant to positive row scaling, the
softmax denominator and the /adj.sum(0) division cancel inside the two
LayerNorms, so the on-chip pipeline is:

  fT    = (feats @ W_v.T).T                  (AllGather #1, bf16, 64KB/rank)
  spre  = (feats_l.T @ adj_l).T @ lin.T      (partial; AllReduce #2, 128KB)
  esT   = exp((f f.T)/8).T  row-shard        (no max-subtract needed)
  dT    = LN_h(esT.T @ f).T * ln2w + ln2b    (partition stats via ones-matmuls)
  sT    = LN_h(spre).T * ln1w + ln1b         (batched 3D-AP LN)
  ta    = exp((2(w*s).T d - dd)/800 - (ss+b)/800)   (E x n_local, e on parts)
  DV    = 1.ta (local), DE = ta.1 (AllReduce #3, 4KB)
  BT    = ta * invDV[col]   (AllGather #4, 1MB)
  AT    = BT * 0.01*invDE[row]
  out   = 0.99*G + AT.T @ BT_full            (G pre-scaled by 0.99 on host)
"""
import numpy as np
import ml_dtypes

import concourse.bass as bass
import concourse.bacc as bacc
import concourse.mybir as mybir
from concourse import tile
from concourse.bass_utils import run_bass_kernel_spmd

BF = ml_dtypes.bfloat16
F32 = np.float32
DT_BF = mybir.dt.bfloat16
DT_F32 = mybir.dt.float32
MULT = mybir.AluOpType.mult
ADD = mybir.AluOpType.add
EXP = mybir.ActivationFunctionType.Exp
SQRT = mybir.ActivationFunctionType.Sqrt
SQUARE = mybir.ActivationFunctionType.Square
IDENT_F = mybir.ActivationFunctionType.Identity

N, E, D, H = 4096, 1024, 256, 64
NC = 8          # cores
NL = N // NC    # 512 local rows
P = 128
NKT = NL // P   # 4  local-row partition tiles
EKT = E // P    # 8  e-chunks
DKT = D // P    # 2  d-chunks
KT = N // P     # 32 n' tiles
NB = 512        # psum column block
NBT = N // NB   # 8
GB = 1024       # G/out dma chunk width
GBT = N // GB   # 4

LN_EPS = 1e-5


def build_kernel(debug_taps=False):
    nc = bacc.Bacc("TRN2", target_bir_lowering=False, debug=False,
                   num_devices=NC)

    # ---- per-core external I/O -------------------------------------------
    adj_e = nc.dram_tensor("adj", [NL, E], DT_BF, kind="ExternalInput")
    g_e = nc.dram_tensor("g", [NL, N], DT_F32, kind="ExternalInput")
    feats_e = nc.dram_tensor("feats", [NL, D], DT_BF, kind="ExternalInput")
    featsT_e = nc.dram_tensor("featsT", [D, NL], DT_BF, kind="ExternalInput")
    wvT_e = nc.dram_tensor("wvT", [D, H], DT_BF, kind="ExternalInput")
    linT_e = nc.dram_tensor("linT", [D, H], DT_BF, kind="ExternalInput")
    wcol_e = nc.dram_tensor("wcol", [H, 1], DT_BF, kind="ExternalInput")
    w2col_e = nc.dram_tensor("w2col", [H, 1], DT_F32, kind="ExternalInput")
    ln1_e = nc.dram_tensor("ln1", [H, 2], DT_F32, kind="ExternalInput")
    ln2_e = nc.dram_tensor("ln2", [H, 2], DT_F32, kind="ExternalInput")
    negb_e = nc.dram_tensor("negb800", [P, 1], DT_F32, kind="ExternalInput")
    ident_e = nc.dram_tensor("ident", [P, P], DT_BF, kind="ExternalInput")
    out_e = nc.dram_tensor("out", [NL, N], DT_F32, kind="ExternalOutput")

    # ---- internal DRAM (collective bounce buffers) -----------------------
    agf_in = nc.dram_tensor("agf_in", [H, NL], DT_BF)
    agf_out = nc.dram_tensor("agf_out", [NC, H, NL], DT_BF, addr_space="Shared")
    ars_in = nc.dram_tensor("ars_in", [P, EKT * H], DT_BF)
    ars_out = nc.dram_tensor("ars_out", [P, EKT * H], DT_BF,
                             addr_space="Shared")
    arde_in = nc.dram_tensor("arde_in", [P, EKT], DT_F32)
    arde_out = nc.dram_tensor("arde_out", [P, EKT], DT_F32, addr_space="Shared")
    agb_in = nc.dram_tensor("agb_in", [EKT, P, NL], DT_BF)
    agb_out = nc.dram_tensor("agb_out", [NC, EKT, P, NL], DT_BF,
                             addr_space="Shared")

    rg = [list(range(NC))]

    with tile.TileContext(nc) as tc:
        with (
            tc.tile_pool(name="pers", bufs=1) as pers,
            tc.tile_pool(name="gio", bufs=1) as gio,
        ):
            def ptile(shape, dt, tag, bufs=None):
                return pers.tile(shape, dt, tag=tag, name=tag, bufs=bufs)

            # ---- load inputs into SBUF ----------------------------------
            featsT_sb = []
            wvT_sb = []
            linT_sb = []
            for k in range(DKT):
                t = ptile([P, NL], DT_BF, f"featsT{k}")
                nc.sync.dma_start(out=t[:], in_=featsT_e[k * P:(k + 1) * P, :])
                featsT_sb.append(t)
                t = ptile([P, H], DT_BF, f"wvT{k}")
                nc.sync.dma_start(out=t[:], in_=wvT_e[k * P:(k + 1) * P, :])
                wvT_sb.append(t)
                t = ptile([P, H], DT_BF, f"linT{k}")
                nc.sync.dma_start(out=t[:], in_=linT_e[k * P:(k + 1) * P, :])
                linT_sb.append(t)
            adj_sb = []
            feats_sb = []
            for k in range(NKT):
                t = ptile([P, E], DT_BF, f"adj{k}")
                nc.sync.dma_start(out=t[:], in_=adj_e[k * P:(k + 1) * P, :])
                adj_sb.append(t)
                t = ptile([P, D], DT_BF, f"feats{k}")
                nc.sync.dma_start(out=t[:], in_=feats_e[k * P:(k + 1) * P, :])
                feats_sb.append(t)
            wcol = ptile([H, 1], DT_BF, "wcol")
            nc.sync.dma_start(out=wcol[:], in_=wcol_e[:, :])
            w2col = ptile([H, 1], DT_F32, "w2col")
            nc.sync.dma_start(out=w2col[:], in_=w2col_e[:, :])
            ln1 = ptile([H, 2], DT_F32, "ln1")
            nc.sync.dma_start(out=ln1[:], in_=ln1_e[:, :])
            ln2 = ptile([H, 2], DT_F32, "ln2")
            nc.sync.dma_start(out=ln2[:], in_=ln2_e[:, :])
            negb = ptile([P, 1], DT_F32, "negb")
            nc.sync.dma_start(out=negb[:], in_=negb_e[:, :])
            ident = ptile([P, P], DT_BF, "ident")
            nc.sync.dma_start(out=ident[:], in_=ident_e[:, :])

            ones_col = ptile([P, 1], DT_BF, "ones_col")
            nc.vector.memset(ones_col[:], 1.0)
            neg_row = ptile([1, P], DT_BF, "neg_row")
            nc.vector.memset(neg_row[:], -1.0)
            eps_col = ptile([P, 1], DT_F32, "eps_col")
            nc.vector.memset(eps_col[:], LN_EPS)

            with (
                tc.tile_pool(name="psA1", bufs=1, space="PSUM") as psA1,
                tc.tile_pool(name="psA2", bufs=1, space="PSUM") as psA2,
                tc.tile_pool(name="escp", bufs=1) as escp,
            ):
                def smtile(shape, dt):
                    return psA1.tile(shape, dt, tag="sm", name="sm", bufs=2)

                # ---- phase 1: fT_loc = wvT.T @ featsT -> AllGather #1 ----
                ps_f = smtile([H, NL], DT_F32)
                for k in range(DKT):
                    nc.tensor.matmul(ps_f[:], lhsT=wvT_sb[k][:],
                                     rhs=featsT_sb[k][:],
                                     start=(k == 0), stop=(k == DKT - 1))
                fT_loc = ptile([H, NL], DT_BF, "fT_loc")
                nc.scalar.copy(fT_loc[:], ps_f[:])
                nc.sync.dma_start(out=agf_in[:, :], in_=fT_loc[:])
                nc.gpsimd.collective_compute(
                    "AllGather", mybir.AluOpType.bypass, replica_groups=rg,
                    ins=[agf_in[:, :]], outs=[agf_out[:, :, :]])

                # ---- phase 2: e_center partials + s_pre partials -> AR #2
                ecs = [[None] * 2 for _ in range(DKT)]
                for dc in range(DKT):
                    for eh in range(2):
                        ps = psA1.tile([P, 512], DT_F32, tag="ec", name="ec",
                                       bufs=1)
                        for k in range(NKT):
                            nc.tensor.matmul(
                                ps[:],
                                lhsT=feats_sb[k][:, dc * P:(dc + 1) * P],
                                rhs=adj_sb[k][:, eh * 512:(eh + 1) * 512],
                                start=(k == 0), stop=(k == NKT - 1))
                        sb = ptile([P, 512], DT_BF, f"ecs{dc}{eh}")
                        nc.scalar.copy(sb[:], ps[:])
                        ecs[dc][eh] = sb
                # s_pre partial: (E, H) = raw_part.T @ linT, e on partitions
                ps_spre = psA1.tile([P, EKT * P], DT_F32, tag="spre",
                                    name="spre", bufs=1)
                for ec in range(EKT):
                    eh, off = ec // 4, (ec % 4) * P
                    for dk in range(DKT):
                        nc.tensor.matmul(
                            ps_spre[:, ec * P:ec * P + H],
                            lhsT=ecs[dk][eh][:, off:off + P],
                            rhs=linT_sb[dk][:],
                            start=(dk == 0), stop=(dk == DKT - 1))
                spre_sb = ptile([P, EKT * H], DT_BF, "spre_sb")
                nc.vector.tensor_copy(
                    spre_sb[:].rearrange("p (a b) -> p a b", b=H),
                    ps_spre[:].rearrange("p (a b) -> p a b", b=P)[:, :, 0:H])
                nc.sync.dma_start(out=ars_in[:, :], in_=spre_sb[:])
                nc.gpsimd.collective_compute(
                    "AllReduce", mybir.AluOpType.add, replica_groups=rg,
                    ins=[ars_in[:, :]], outs=[ars_out[:, :]])

                # ---- phase 3: assemble fT_full; f_nat via PE transpose ---
                fT_full = ptile([H, N], DT_BF, "fT_full")
                nc.sync.dma_start(
                    out=fT_full[:].rearrange("p (r f) -> p r f", r=NC),
                    in_=agf_out[:, :, :].rearrange("r p f -> p r f"))
                f_nat = ptile([P, KT * H], DT_BF, "f_nat")
                for k in range(KT):
                    pt = smtile([P, H], DT_BF)
                    nc.tensor.transpose(pt[:], fT_full[:, k * P:(k + 1) * P],
                                        ident[:H, :H])
                    nc.vector.tensor_copy(f_nat[:, k * H:(k + 1) * H], pt[:])

                # ---- phase 4: expscoresT + dT accumulation ---------------
                ps_dT = psA2.tile([H, NL], DT_F32, tag="dT", name="dT",
                                  bufs=1)
                for k in range(KT):
                    ps = psA2.tile([P, NL], DT_F32, tag="sc", name="sc",
                                   bufs=2)
                    nc.tensor.matmul(ps[:], lhsT=fT_full[:, k * P:(k + 1) * P],
                                     rhs=fT_loc[:], start=True, stop=True)
                    es = escp.tile([P, NL], DT_BF, tag=f"esc{k}",
                                   name=f"esc{k}")
                    nc.scalar.activation(es[:], ps[:], EXP, scale=0.125)
                    nc.tensor.matmul(ps_dT[:],
                                     lhsT=f_nat[:, k * H:(k + 1) * H],
                                     rhs=es[:],
                                     start=(k == 0), stop=(k == KT - 1))

                # dT LayerNorm: stats along partition dim via ones-matmuls
                dT_pre = ptile([H, NL], DT_BF, "dT_pre")
                nc.vector.tensor_copy(dT_pre[:], ps_dT[:])
                d2 = ptile([H, NL], DT_BF, "d2tmp")
                nc.vector.tensor_mul(d2[:], dT_pre[:], dT_pre[:])
                ps_srow = smtile([1, NL], DT_F32)
                nc.tensor.matmul(ps_srow[:], lhsT=ones_col[:H, :],
                                 rhs=dT_pre[:], start=True, stop=True)
                ps_sqrow = smtile([1, NL], DT_F32)
                nc.tensor.matmul(ps_sqrow[:], lhsT=ones_col[:H, :], rhs=d2[:],
                                 start=True, stop=True)
                mean_r = ptile([1, NL], DT_F32, "mean_r")
                nc.scalar.mul(mean_r[:], ps_srow[:], 1.0 / H)
                msq_r = ptile([1, NL], DT_F32, "msq_r")
                nc.vector.tensor_mul(msq_r[:], mean_r[:], mean_r[:])
                var_r = ptile([1, NL], DT_F32, "var_r")
                nc.scalar.mul(var_r[:], ps_sqrow[:], 1.0 / H)
                nc.vector.tensor_sub(var_r[:], var_r[:], msq_r[:])
                sd_r = ptile([1, NL], DT_F32, "sd_r")
                nc.scalar.activation(sd_r[:], var_r[:], SQRT,
                                     bias=eps_col[:1, :])
                rstd_r = ptile([1, NL], DT_F32, "rstd_r")
                nc.vector.reciprocal(rstd_r[:], sd_r[:])
                rstd_bf = ptile([1, NL], DT_BF, "rstd_bf")
                nc.vector.tensor_copy(rstd_bf[:], rstd_r[:])
                nmr_bf = ptile([1, NL], DT_BF, "nmr_bf")
                nc.vector.scalar_tensor_tensor(
                    nmr_bf[:], mean_r[:], -1.0, rstd_r[:], MULT, MULT)
                ab_bc = ptile([H, 2 * NL], DT_BF, "ab_bc")
                nc.gpsimd.partition_broadcast(ab_bc[:, 0:NL], rstd_bf[:])
                nc.gpsimd.partition_broadcast(ab_bc[:, NL:2 * NL], nmr_bf[:])
                t1 = ptile([H, NL], DT_F32, "dnorm_t1")
                nc.vector.tensor_mul(t1[:], dT_pre[:], ab_bc[:, 0:NL])
                nc.vector.tensor_add(t1[:], t1[:], ab_bc[:, NL:2 * NL])
                dT_ln = ptile([H, NL], DT_BF, "dT_ln")
                nc.vector.tensor_scalar(dT_ln[:], t1[:], ln2[:, 0:1],
                                        ln2[:, 1:2], MULT, ADD)
                d2T = ptile([H, NL], DT_BF, "d2T")
                nc.vector.tensor_mul(d2T[:], dT_ln[:], dT_ln[:])
                ps_dd = smtile([1, NL], DT_F32)
                nc.tensor.matmul(ps_dd[:], lhsT=wcol[:], rhs=d2T[:],
                                 start=True, stop=True)
                dd_bf = ptile([1, NL], DT_BF, "dd_bf")
                nc.scalar.copy(dd_bf[:], ps_dd[:])

                # ---- phase 5: s side, batched LN (after AllReduce #2) ----
                spre_r = ptile([P, EKT * H], DT_BF, "spre_r")
                nc.sync.dma_start(out=spre_r[:], in_=ars_out[:, :])
                spre3 = spre_r[:].rearrange("p (a b) -> p a b", b=H)
                sum3 = ptile([P, EKT], DT_F32, "sum3")
                nc.vector.reduce_sum(sum3[:], spre3,
                                     axis=mybir.AxisListType.X)
                nmean3 = ptile([P, EKT], DT_F32, "nmean3")
                nc.scalar.mul(nmean3[:], sum3[:], -1.0 / H)
                xc = ptile([P, EKT * H], DT_F32, "s_xc")
                xc3 = xc[:].rearrange("p (a b) -> p a b", b=H)
                nc.vector.tensor_add(
                    xc3, spre3,
                    nmean3[:].rearrange("p (a b) -> p a b", b=1)
                    .to_broadcast((P, EKT, H)))
                sq = ptile([P, EKT * H], DT_F32, "s_sq")
                sq3 = sq[:].rearrange("p (a b) -> p a b", b=H)
                nc.vector.tensor_mul(sq3, xc3, xc3)
                vs3 = ptile([P, EKT], DT_F32, "vs3")
                nc.vector.reduce_sum(vs3[:], sq3, axis=mybir.AxisListType.X)
                sd3 = ptile([P, EKT], DT_F32, "sd3")
                nc.scalar.activation(sd3[:], vs3[:], SQRT, scale=1.0 / H,
                                     bias=eps_col[:])
                rstd3 = ptile([P, EKT], DT_F32, "rstd3")
                nc.vector.reciprocal(rstd3[:], sd3[:])
                snrm = ptile([P, EKT * H], DT_BF, "snrm")
                nc.vector.tensor_mul(
                    snrm[:].rearrange("p (a b) -> p a b", b=H), xc3,
                    rstd3[:].rearrange("p (a b) -> p a b", b=1)
                    .to_broadcast((P, EKT, H)))
                sT_nrm = ptile([H, E], DT_BF, "sT_nrm")
                for ec in range(EKT):
                    pt = psA2.tile([H, P], DT_BF, tag="sc", name="sc", bufs=2)
                    nc.tensor.transpose(pt[:],
                                        snrm[:, ec * H:(ec + 1) * H],
                                        ident[:])
                    nc.vector.tensor_copy(sT_nrm[:, ec * P:(ec + 1) * P],
                                          pt[:])
                sT_ln = ptile([H, E], DT_BF, "sT_ln")
                nc.vector.tensor_scalar(sT_ln[:], sT_nrm[:], ln1[:, 0:1],
                                        ln1[:, 1:2], MULT, ADD)

            sT2w = ptile([H, E], DT_BF, "sT2w")
            nc.vector.tensor_scalar(sT2w[:], sT_ln[:], w2col[:], None, MULT)
            s2T = ptile([H, E], DT_BF, "s2T")
            nc.vector.tensor_mul(s2T[:], sT_ln[:], sT_ln[:])

            # ---- phase 6: ta tiles, DV, DE ------------------------------
            with tc.tile_pool(name="psB", bufs=1, space="PSUM") as psB:
                bias_sb = ptile([P, EKT], DT_F32, "bias_sb")
                de_cols = ptile([P, EKT], DT_F32, "de_cols")
                ta_all = ptile([P, EKT * NL], DT_BF, "ta_all")
                for ec in range(EKT):
                    ps_ss = psB.tile([P, 1], DT_F32, tag="ss", name="ss",
                                     bufs=2)
                    nc.tensor.matmul(ps_ss[:],
                                     lhsT=s2T[:, ec * P:(ec + 1) * P],
                                     rhs=wcol[:], start=True, stop=True)
                    nc.vector.scalar_tensor_tensor(
                        bias_sb[:, ec:ec + 1], ps_ss[:], -1.0 / 800.0,
                        negb[:], MULT, ADD)
                    ps = psB.tile([P, NL], DT_F32, tag="ta", name="ta",
                                  bufs=2)
                    nc.tensor.matmul(ps[:], lhsT=sT2w[:, ec * P:(ec + 1) * P],
                                     rhs=dT_ln[:], start=True, stop=False)
                    nc.tensor.matmul(ps[:], lhsT=neg_row[:], rhs=dd_bf[:],
                                     start=False, stop=True)
                    nc.scalar.activation(ta_all[:, ec * NL:(ec + 1) * NL],
                                         ps[:], EXP, scale=1.0 / 800.0,
                                         bias=bias_sb[:, ec:ec + 1],
                                         accum_out=de_cols[:, ec:ec + 1])

                # DE AllReduce (#3) — overlaps BT build below
                nc.sync.dma_start(out=arde_in[:, :], in_=de_cols[:])
                nc.gpsimd.collective_compute(
                    "AllReduce", mybir.AluOpType.add, replica_groups=rg,
                    ins=[arde_in[:, :]], outs=[arde_out[:, :]])

                # DV (local): column sums over all e -> invDV broadcast
                ps_dv = psB.tile([1, NL], DT_F32, tag="dv", name="dv", bufs=1)
                for ec in range(EKT):
                    nc.tensor.matmul(ps_dv[:], lhsT=ones_col[:],
                                     rhs=ta_all[:, ec * NL:(ec + 1) * NL],
                                     start=(ec == 0), stop=(ec == EKT - 1))
                rdv = ptile([1, NL], DT_F32, "rdv")
                nc.vector.reciprocal(rdv[:], ps_dv[:])
                invdv_row = ptile([1, NL], DT_BF, "invdv_row")
                nc.scalar.activation(invdv_row[:], rdv[:], SQRT)
                invdv_bc = ptile([P, NL], DT_BF, "invdv_bc")
                nc.gpsimd.partition_broadcast(invdv_bc[:], invdv_row[:])

                # BT = ta * invDV[col]  (one 3D-broadcast DVE op) -> AG #4
                bt_all = ptile([P, EKT * NL], DT_BF, "bt_all")
                nc.vector.tensor_mul(
                    bt_all[:].rearrange("p (a b) -> p a b", b=NL),
                    ta_all[:].rearrange("p (a b) -> p a b", b=NL),
                    invdv_bc[:].rearrange("p (a b) -> p a b", a=1)
                    .to_broadcast((P, EKT, NL)))
                nc.sync.dma_start(
                    out=agb_in[:, :, :].rearrange("a p b -> p a b"),
                    in_=bt_all[:].rearrange("p (a b) -> p a b", b=NL))
                nc.gpsimd.collective_compute(
                    "AllGather", mybir.AluOpType.bypass, replica_groups=rg,
                    ins=[agb_in[:, :, :]], outs=[agb_out[:, :, :, :]])

                # invDE from AllReduce; AT = BT * (0.01*invDE)
                de_sb = ptile([P, EKT], DT_F32, "de_sb")
                nc.sync.dma_start(out=de_sb[:], in_=arde_out[:, :])
                invde = ptile([P, EKT], DT_F32, "invde")
                nc.vector.reciprocal(invde[:], de_sb[:])
                invde01 = ptile([P, EKT], DT_BF, "invde01")
                nc.vector.tensor_scalar(invde01[:], invde[:], 0.01, None,
                                        MULT)
                at_all = ptile([P, EKT * NL], DT_BF, "at_all")
                nc.vector.tensor_mul(
                    at_all[:].rearrange("p (a b) -> p a b", b=NL),
                    bt_all[:].rearrange("p (a b) -> p a b", b=NL),
                    invde01[:].rearrange("p (a b) -> p a b", b=1)
                    .to_broadcast((P, EKT, NL)))

            # ---- phase 7: big matmul + epilogue -------------------------
            with (
                tc.tile_pool(name="psC", bufs=8, space="PSUM") as psC,
                tc.tile_pool(name="btfp", bufs=1) as btfp,
            ):
                btf = []
                for k in range(EKT):
                    t = btfp.tile([P, N], DT_BF, tag=f"btf{k}", name=f"btf{k}")
                    nc.sync.dma_start(
                        out=t[:].rearrange("p (r f) -> p r f", r=NC),
                        in_=agb_out[:, k, :, :].rearrange("r p f -> p r f"))
                    btf.append(t)

                for m in range(NKT):
                    gsbs = []
                    for gc in range(GBT):
                        gsb = gio.tile([P, GB], DT_F32, tag="gsb", name="gsb",
                                       bufs=5)
                        nc.sync.dma_start(
                            out=gsb[:],
                            in_=g_e[m * P:(m + 1) * P, gc * GB:(gc + 1) * GB])
                        gsbs.append(gsb)
                    pss = []
                    for nb in range(NBT):
                        pss.append(psC.tile([P, NB], DT_F32, tag="big",
                                            name="big"))
                    for k in range(EKT):
                        for nb in range(NBT):
                            nc.tensor.matmul(
                                pss[nb][:],
                                lhsT=at_all[:, k * NL + m * P:
                                            k * NL + (m + 1) * P],
                                rhs=btf[k][:, nb * NB:(nb + 1) * NB],
                                start=(k == 0), stop=(k == EKT - 1))
                    for gc in range(GBT):
                        osb = gio.tile([P, GB], DT_F32, tag="osb", name="osb",
                                       bufs=3)
                        for h in range(2):
                            nb = gc * 2 + h
                            nc.vector.tensor_add(
                                osb[:, h * NB:(h + 1) * NB],
                                gsbs[gc][:, h * NB:(h + 1) * NB],
                                pss[nb][:])
                        nc.sync.dma_start(
                            out=out_e[m * P:(m + 1) * P,
                                      gc * GB:(gc + 1) * GB],
                            in_=osb[:])


            if debug_taps:
                for bk, bt_t in enumerate(btf):
                    ext = nc.dram_tensor(f"d_btf{bk}", list(bt_t.shape),
                                         bt_t.dtype, kind="ExternalOutput")
                    nc.sync.dma_start(out=ext[...], in_=bt_t[:])
                taps = {
                    "d_fT_loc": fT_loc, "d_fT_full": fT_full,
                    "d_f_nat": f_nat, "d_spre_sb": spre_sb,
                    "d_spre_r": spre_r, "d_snrm": snrm,
                    "d_sT_ln": sT_ln, "d_sT2w": sT2w, "d_s2T": s2T,
                    "d_dT_pre": dT_pre, "d_dT_ln": dT_ln,
                    "d_dd_bf": dd_bf, "d_bias_sb": bias_sb,
                    "d_ta_all": ta_all, "d_de_sb": de_sb, "d_de_cols": de_cols,
                    "d_invdv_row": invdv_row, "d_bt_all": bt_all,
                    "d_at_all": at_all, "d_ab_bc": ab_bc,
                }
                for nm, t in taps.items():
                    ext = nc.dram_tensor(nm, list(t.shape), t.dtype,
                                         kind="ExternalOutput")
                    nc.sync.dma_start(out=ext[...], in_=t[:])

    nc.compile()
    return nc


_NC_CACHE = None


def _get_nc():
    global _NC_CACHE
    if _NC_CACHE is None:
        _NC_CACHE = build_kernel()
    return _NC_CACHE


def make_in_maps(adj, G, feats, W_v_w, lin_w, w_o_w, w_o_b,
                 ln1_w, ln1_b, ln2_w, ln2_b, kn=None):
    adj = np.asarray(adj, F32)
    G = np.asarray(G, F32)
    feats = np.asarray(feats, F32)
    W_v_w = np.asarray(W_v_w, F32)
    lin_w = np.asarray(lin_w, F32)
    w = np.asarray(w_o_w, F32)[0]
    b = float(np.asarray(w_o_b, F32).reshape(-1)[0])
    ln1_w = np.asarray(ln1_w, F32).reshape(-1)
    ln1_b = np.asarray(ln1_b, F32).reshape(-1)
    ln2_w = np.asarray(ln2_w, F32).reshape(-1)
    ln2_b = np.asarray(ln2_b, F32).reshape(-1)

    g99 = G * np.float32(0.99)
    adj_bf = adj.astype(BF)
    feats_bf = feats.astype(BF)
    featsT_bf = np.ascontiguousarray(feats.T).astype(BF)
    wvT = np.ascontiguousarray(W_v_w.T).astype(BF)
    linT = np.ascontiguousarray(lin_w.T).astype(BF)
    wcol = np.ascontiguousarray(w.reshape(H, 1)).astype(BF)
    w2col = np.ascontiguousarray((2.0 * w).reshape(H, 1)).astype(F32)
    ln1 = np.stack([ln1_w, ln1_b], axis=1).astype(F32)
    ln2 = np.stack([ln2_w, ln2_b], axis=1).astype(F32)
    negb = np.full((P, 1), -b / 800.0, F32)
    ident = np.eye(P, dtype=BF)

    in_maps = []
    for i in range(NC):
        sl = slice(i * NL, (i + 1) * NL)
        in_maps.append({
            "adj": np.ascontiguousarray(adj_bf[sl]),
            "g": np.ascontiguousarray(g99[sl]),
            "feats": np.ascontiguousarray(feats_bf[sl]),
            "featsT": np.ascontiguousarray(featsT_bf[:, sl]),
            "wvT": wvT,
            "linT": linT,
            "wcol": wcol,
            "w2col": w2col,
            "ln1": ln1,
            "ln2": ln2,
            "negb800": negb,
            "ident": ident,
        })
    return in_maps


def kernel(**inputs) -> np.ndarray:
    nc = _get_nc()
    in_maps = make_in_maps(**inputs)
    res = run_bass_kernel_spmd(nc, in_maps, core_ids=list(range(NC))).results
    return np.concatenate([np.asarray(res[i]["out"]) for i in range(NC)],
                          axis=0)


if __name__ == "__main__":
    import reference
    inputs = reference.setup_inputs()
    out = kernel(**{k: np.asarray(v) if not np.isscalar(v) else v
                    for k, v in inputs.items()})
    print("out", out.shape, out.dtype)
